# revision 14
# baseline (speedup 1.0000x reference)
"""DecoderRNN Trainium2 kernel (v4).

Math (reference):
    emb = embed_table[captions]                      # (B, 31, E)
    inputs = concat([features[:,None,:], emb], 1)    # (B, T=32, E)
    xproj = inputs @ Wi + bi                         # (B, T, H)
    h_t = tanh(xproj_t + h_{t-1} @ Wh + bh)          # scan over T
    out = hs @ Wy + by                               # (B, T, V)

Distribution: vocab-parallel output projection across 8 cores (Wy/by sharded
by 1250 columns); the embedding gather, input GEMM and serial RNN are
replicated full-batch on every core. No collectives.

Structure (driven by HW traces of v1..v3):
  - Per-block front-end pipeline: each gathered 128-row block (= 2
    timesteps, t-major) flows gather -> PE-transpose -> 16-matmul GEMM ->
    one DVE bias-add into xpT -> unlocks two more RNN steps.  The RNN rides
    the gather stream instead of waiting for 512-row chunks.
  - Step 1 needs only features @ Wi: features arrive as a direct (host-
    transposed) input, so the recurrence starts ~flat after the weights
    land, with no gather dependency.
  - RNN step: one [128,256] PSUM tile, one identity-stationary inject of
    xpT[t-1] (start=True, runs early: its bank was freed at t-2), 16 Wh
    matmuls accumulate, one tanh reads PSUM and writes all four h-chunks.
    The inject on the PE avoids the cross-engine PSUM write hazard that a
    DVE prefill hits on hardware.
  - The PE clock has p-states (~2.4GHz solid / ~1.2GHz choppy): no
    projection work inside the choppy RNN phase; the projection runs as one
    solid block. Its PSUM->SBUF drain alternates between DVE (tensor_add
    with fp32 by) and PE-inject-of-by + scalar-engine copy, so no single
    engine bottlenecks the phase.
  - Weights host-pre-arranged for contiguous DMAs; wy/by ride the scalar
    queue behind the first GEMM writes so they stay off the gather window.
  - Output fp16 (quantization ~1e-4 of the fp32 envelope), one DMA per
    128-row tile.
"""

import sys

sys.path.insert(0, "/opt/trn_rl_repo")

from contextlib import ExitStack

import numpy as np

import concourse.bass as bass
import concourse.mybir as mybir
import concourse.tile as tile
from concourse import bacc
from concourse.bass import ts
from concourse.bass_utils import run_bass_kernel_spmd
from concourse.masks import make_identity

B, T, E, H, V = 64, 32, 512, 512, 10000
NCORES = 8
VS = V // NCORES          # vocab shard per core
BT = B * T                # 2048 rows, t-major: row = t*64 + b
P = 128
KE = E // P               # 4 contraction chunks over E
KH = H // P               # 4 contraction chunks over H
MT = H // P               # 4 output chunks of H
NBT = BT // P             # 16 bt blocks (2 timesteps each)
F32 = mybir.dt.float32
F16 = mybir.dt.float16

# projection N-chunks (psum bank holds 512 fp32 per partition)
VCHUNKS = [(0, 512), (512, 512), (1024, 226)]
assert sum(n for _, n in VCHUNKS) == VS


def build_program() -> bass.Bass:
    nc = bacc.Bacc()

    aug = nc.dram_tensor("aug_table", [V + B, E], F16, kind="ExternalInput")
    idx = nc.dram_tensor("idx", [P, NBT], mybir.dt.int32, kind="ExternalInput")
    featT = nc.dram_tensor("featT", [P, KE * B], F16, kind="ExternalInput")
    wi = nc.dram_tensor("wi", [P, KE * H], F16, kind="ExternalInput")
    wh = nc.dram_tensor("wh", [P, KH * MT * P], F16, kind="ExternalInput")
    bias = nc.dram_tensor("bias", [P, MT], F32, kind="ExternalInput")  # bi+bh
    wy = nc.dram_tensor("wy", [P, KH * VS], F16, kind="ExternalInput")
    byr = nc.dram_tensor("byr", [P, VS], F32, kind="ExternalInput")
    byr16 = nc.dram_tensor("byr16", [P, VS], F16, kind="ExternalInput")
    out = nc.dram_tensor("out", [BT, VS], F16, kind="ExternalOutput")

    with ExitStack() as ctx:
        tc = ctx.enter_context(tile.TileContext(nc))
        persist = ctx.enter_context(tc.tile_pool(name="persist", bufs=1))
        osb_pool = ctx.enter_context(tc.tile_pool(name="osb", bufs=2))
        tp_psum = ctx.enter_context(tc.tile_pool(name="tp_ps", bufs=2, space="PSUM"))
        gm_psum = ctx.enter_context(tc.tile_pool(name="gm_ps", bufs=3, space="PSUM"))
        rn_psum = ctx.enter_context(tc.tile_pool(name="rn_ps", bufs=3, space="PSUM"))

        # ---- idx first: it gates the gather, the kernel's critical early path
        idx_sb = persist.tile([P, NBT], mybir.dt.int32, tag="idx")
        nc.sync.dma_start(out=idx_sb[:], in_=idx[:])
        ident = persist.tile([P, P], F16, tag="ident")
        make_identity(nc, ident[:])

        # ---- early loads (contiguous; only what the front end needs)
        featT_sb = persist.tile([P, KE, B], F16, tag="featT")
        nc.sync.dma_start(
            out=featT_sb[:], in_=featT[:].rearrange("p (k b) -> p k b", k=KE)
        )
        wi_sb = persist.tile([P, KE, H], F16, tag="wi")
        nc.sync.dma_start(out=wi_sb[:], in_=wi[:].rearrange("p (k h) -> p k h", k=KE))
        wh_sb = persist.tile([P, KH, MT, P], F16, tag="wh")
        nc.sync.dma_start(
            out=wh_sb[:], in_=wh[:].rearrange("p (k m q) -> p k m q", k=KH, m=MT)
        )
        bias_sb = persist.tile([P, MT], F32, tag="bias")
        nc.sync.dma_start(out=bias_sb[:], in_=bias[:])

        # ---- persistent activations
        nat = persist.tile([P, NBT, E], F16, tag="nat")          # gathered rows
        inputsT = persist.tile([P, KE, BT], F16, tag="inputsT")  # E-on-partitions
        xpT = persist.tile([P, T, MT * B], F16, tag="xpT")       # xproj + bias
        hsT = persist.tile([P, MT, (T + 1) * B], F16, tag="hsT") # h states, slot0=0
        wy_sb = persist.tile([P, KH, VS], F16, tag="wy")
        by_rep = persist.tile([P, VS], F32, tag="by_rep")
        by16_sb = persist.tile([P, VS], F16, tag="by16")
        nc.vector.memset(hsT[:, :, 0:B], 0.0)

        # ---- all 16 indirect gathers up front on gpsimd (software DGE)
        for i in range(NBT):
            nc.gpsimd.indirect_dma_start(
                out=nat[:, i, :],
                out_offset=None,
                in_=aug[:],
                in_offset=bass.IndirectOffsetOnAxis(ap=idx_sb[:, i : i + 1], axis=0),
            )

        # bias broadcast AP: [P, tn, MT, B] view of bias_sb [P, MT] with
        # stride 0 over t and b
        def bias_bc(tn):
            b_ap = bias_sb[:]
            pstr, pn = b_ap.ap[0]
            return bass.AP(
                tensor=b_ap.tensor, offset=b_ap.offset,
                ap=[[pstr, pn], [0, tn], [1, MT], [0, B]],
            )

        def emit_xp_write(ps_view, t0, tn):
            # xpT[:, t0:t0+tn, :] = psum + (bi + bh), one DVE op
            nc.vector.tensor_add(
                xpT[:, t0 : t0 + tn, :].rearrange("p t (m b) -> p t m b", b=B),
                ps_view,
                bias_bc(tn),
            )

        def emit_feat_gemm():
            # xpT[:, 0, :] = (features @ Wi).T + bias   (no gather needed)
            ps = gm_psum.tile([P, 512], F32, tag="mm")
            for m in range(MT):
                for k in range(KE):
                    nc.tensor.matmul(
                        ps[:, ts(m, B)],
                        lhsT=wi_sb[:, k, ts(m, P)],
                        rhs=featT_sb[:, k, :],
                        start=(k == 0),
                        stop=(k == KE - 1),
                    )
            emit_xp_write(
                ps[:, 0:256].rearrange("p (t m b) -> p t m b", t=1, b=B), 0, 1
            )

        def emit_block(i):
            # gathered block i (timesteps 2i, 2i+1): PE-transpose 4 e-chunks,
            # 16-matmul GEMM, one DVE bias-add into xpT
            for k in range(KE):
                tp = tp_psum.tile([P, P], F16, tag="tp")
                nc.tensor.matmul(
                    tp[:], lhsT=nat[:, i, ts(k, P)], rhs=ident[:], is_transpose=True,
                )
                if k % 2 == 0:
                    nc.vector.tensor_copy(inputsT[:, k, ts(i, P)], tp[:])
                else:
                    nc.scalar.activation(
                        inputsT[:, k, ts(i, P)], tp[:],
                        mybir.ActivationFunctionType.Identity,
                    )
            ps = gm_psum.tile([P, 512], F32, tag="mm")
            for m in range(MT):
                for k in range(KE):
                    nc.tensor.matmul(
                        ps[:, ts(m, P)],
                        lhsT=wi_sb[:, k, ts(m, P)],
                        rhs=inputsT[:, k, ts(i, P)],
                        start=(k == 0),
                        stop=(k == KE - 1),
                    )
            v = ps[:].rearrange("p (m t b) -> p t m b", t=2, b=B)
            if i == 0:
                emit_xp_write(v[:, 1:2], 1, 1)  # t=0 already done from features
            else:
                emit_xp_write(v, 2 * i, 2)

        def emit_step(t):
            # h_t = tanh(xpT[t-1] + Wh.T @ h_{t-1}) in one [128,256] PSUM
            # tile; xpT injected by the PE (identity stationary, start=True)
            rp = rn_psum.tile([P, MT * B], F32, tag="rnn")
            nc.tensor.matmul(
                rp[:], lhsT=ident[:], rhs=xpT[:, t - 1, :],
                start=True, stop=False, skip_group_check=True,
            )
            for k in range(KH):
                for m in range(MT):
                    nc.tensor.matmul(
                        rp[:, ts(m, B)],
                        lhsT=wh_sb[:, k, m, :],
                        rhs=hsT[:, k, (t - 1) * B : t * B],
                        start=False,
                        stop=(k == KH - 1),
                        skip_group_check=True,
                    )
            nc.scalar.activation(
                hsT[:, :, t * B : (t + 1) * B],
                rp[:].rearrange("p (m b) -> p m b", b=B),
                mybir.ActivationFunctionType.Tanh,
            )

        # ---- interleaved front end + RNN: block i unlocks steps 2i+1, 2i+2
        emit_feat_gemm()
        emit_step(1)
        # wy/by ride the scalar queue behind early work, off the gather's
        # DMA window but well before the projection needs them
        nc.scalar.dma_start(out=wy_sb[:], in_=wy[:].rearrange("p (k v) -> p k v", k=KH))
        nc.scalar.dma_start(out=by_rep[:], in_=byr[:])
        nc.scalar.dma_start(out=by16_sb[:], in_=byr16[:])
        for i in range(NBT):
            emit_block(i)
            if i == 0:
                emit_step(2)
            else:
                emit_step(2 * i + 1)
                emit_step(2 * i + 2)

        # ---- projection: one solid PE block (clock ramps); psum drain
        # alternates DVE tensor_add(+by) and PE-inject-by + scalar copy
        osb_tiles = {}
        for gi, (i, vc) in enumerate(
            (i, vc) for i in range(NBT) for vc in range(len(VCHUNKS))
        ):
            v0, vn = VCHUNKS[vc]
            if vc == 0:
                osb = osb_pool.tile([P, VS], F16, tag="osb")
                osb_tiles[i] = osb
            pp = gm_psum.tile([P, 512], F32, tag="mm")
            inject = gi % 2 == 1
            if inject:
                nc.tensor.matmul(
                    pp[:, :vn], lhsT=ident[:], rhs=by16_sb[:, v0 : v0 + vn],
                    start=True, stop=False, skip_group_check=True,
                )
            for k in range(KH):
                nc.tensor.matmul(
                    pp[:, :vn],
                    lhsT=hsT[:, k, (2 * i + 1) * B : (2 * i + 1) * B + P],
                    rhs=wy_sb[:, k, v0 : v0 + vn],
                    start=(k == 0 and not inject),
                    stop=(k == KH - 1),
                    skip_group_check=True,
                )
            osb = osb_tiles[i]
            if inject:
                nc.scalar.activation(
                    osb[:, v0 : v0 + vn], pp[:, :vn],
                    mybir.ActivationFunctionType.Identity,
                )
            else:
                nc.vector.tensor_add(
                    osb[:, v0 : v0 + vn], pp[:, :vn], by_rep[:, v0 : v0 + vn]
                )
            if vc == len(VCHUNKS) - 1:
                nc.sync.dma_start(out=out[ts(i, P), :], in_=osb[:])
                del osb_tiles[i]

    nc.compile()
    return nc


def make_in_maps(features, captions, embed_table, Wi, bi, Wh, bh, Wy, by):
    f32, f16 = np.float32, np.float16
    aug = np.concatenate(
        [np.asarray(embed_table, f32), np.asarray(features, f32)], axis=0
    ).astype(f16)
    idx = np.empty((T, B), np.int32)
    idx[0] = V + np.arange(B, dtype=np.int32)
    idx[1:] = np.asarray(captions, np.int64).T.astype(np.int32)
    idx_t = np.ascontiguousarray(idx.reshape(BT).reshape(NBT, P).T)  # [128, 16]

    feat16 = np.asarray(features, f32).astype(f16)  # [B, E]
    featT_h = np.ascontiguousarray(
        feat16.T.reshape(KE, P, B).transpose(1, 0, 2).reshape(P, KE * B)
    )

    # host pre-arrangement: weight DMAs become one contiguous run/partition
    wi16 = np.asarray(Wi, f32).astype(f16)      # [E, H]
    wi_h = np.ascontiguousarray(
        wi16.reshape(KE, P, H).transpose(1, 0, 2).reshape(P, KE * H)
    )
    wh16 = np.asarray(Wh, f32).astype(f16)      # [H, H]
    wh_h = np.ascontiguousarray(
        wh16.reshape(KH, P, MT, P).transpose(1, 0, 2, 3).reshape(P, KH * MT * P)
    )
    bias_c = (np.asarray(bi, f32) + np.asarray(bh, f32)).astype(f32)
    bias_h = np.ascontiguousarray(bias_c.reshape(MT, P).T)  # [128, MT]
    wy16 = np.asarray(Wy, f32).astype(f16)      # [H, V]
    by_f = np.asarray(by, f32)

    in_maps = []
    for c in range(NCORES):
        wy_sh = wy16[:, c * VS : (c + 1) * VS]  # [H, VS]
        wy_h = np.ascontiguousarray(
            wy_sh.reshape(KH, P, VS).transpose(1, 0, 2).reshape(P, KH * VS)
        )
        by_sh = by_f[c * VS : (c + 1) * VS]
        byr_h = np.ascontiguousarray(np.broadcast_to(by_sh, (P, VS)))
        in_maps.append(
            {
                "aug_table": aug,
                "idx": idx_t,
                "featT": featT_h,
                "wi": wi_h,
                "wh": wh_h,
                "bias": bias_h,
                "wy": wy_h,
                "byr": byr_h,
                "byr16": byr_h.astype(f16),
            }
        )
    return in_maps


def assemble(core_outs):
    full = np.concatenate([np.asarray(o) for o in core_outs], axis=1)  # [BT, V]
    return np.ascontiguousarray(
        full.reshape(T, B, V).transpose(1, 0, 2).astype(np.float32)
    )


def kernel(**inputs) -> np.ndarray:
    in_maps = make_in_maps(**inputs)
    nc = build_program()
    res = run_bass_kernel_spmd(nc, in_maps, core_ids=list(range(NCORES)))
    return assemble([r["out"] for r in res.results])
